# revision 13
# baseline (speedup 1.0000x reference)
"""BIOUL-constrained CRF NLL on 8 Trainium2 NeuronCores.

Reference computation: mean over batch of (gold path score - log partition Z)
for a linear-chain CRF with BIOUL transition constraints.
  emissions [1024,1024,41] f32, mask [1024,1024] bool (contiguous lengths),
  tags [1024,1024] int, transitions [41,41], start/end transitions [41].

Device strategy (data parallel: 128 batch lanes per core, organized as
2 pipeline groups x 2 vertically-packed chains x 32 lanes):
  The forward logsumexp scan runs in scaled-exp space, so each step is one
  TensorEngine matmul plus one vector multiply:
    A_t[j,b] = (sum_i A_{t-1}[i,b] * E[i,j]) * exp(em[t,j,b])
  with E = exp(constrained transitions) (forbidden entries exactly 0).
  Two chains are stacked on the partition axis (rows 0..40 and 42..82) and
  share one block-diagonal stationary matrix; its columns 96/97 also produce
  endsum(A) = sum_j A[j,b]*exp(end[j]) for both chains, which is streamed out
  every step (z = log(endsum) at t = len-1) and doubles as the periodic
  rescaling divisor (every 8 steps, computed 4 steps ahead of application so
  the reciprocal/broadcast sit off the serial critical path).  The host does
  the cheap parts: input transpose, gold-path score (gathers), the log/cumsum
  bookkeeping of the rescales, and the final mean.
"""

import numpy as np

IMPOSSIBLE = -10000.0
NUM_LABELS = 10
K = 41
B = 1024
T = 1024
NCORES = 8
BLOC = B // NCORES          # 128 batch lanes per core
NG = 2                      # independent pipeline groups (latency hiding)
NV = 2                      # chains stacked on partitions per group
BC = 32                     # lanes per chain
ROWS = 2 * K + 1            # 83: chain0 rows 0..40, pad row 41, chain1 42..82
MAUG = 98                   # stationary free size: cols 96/97 = endsums
AUX0 = 96                   # aligned aux partition base
KSTEP = 8                   # steps per PSUM block
RESCALE_EVERY = 32          # rescale period (steps)
NBLK = T // KSTEP           # 128

_CACHE = {}
PS_BUFS = 2
PSR_BUFS = 2
STATE_BUFS = 3
DO_RESCALE = True
MU = 2.8


def _bioul_masks():
    O, Bt, I, L, U = 0, 1, 2, 3, 4
    k = 1 + 4 * NUM_LABELS
    tmask = np.ones((k, k), dtype=bool)
    tmask[O, O] = 0
    for i in range(NUM_LABELS):
        S = 4 * i
        tmask[O, Bt + S] = 0
        tmask[Bt + S, I + S] = 0
        tmask[I + S, I + S] = 0
        tmask[I + S, L + S] = 0
        tmask[Bt + S, L + S] = 0
        tmask[L + S, O] = 0
        tmask[O, U + S] = 0
        tmask[U + S, O] = 0
        for j in range(NUM_LABELS):
            SJ = 4 * j
            tmask[L + S, Bt + SJ] = 0
            tmask[L + S, U + SJ] = 0
            tmask[U + S, Bt + SJ] = 0
    smask = np.zeros(k, dtype=bool)
    emask = np.zeros(k, dtype=bool)
    for i in range(NUM_LABELS):
        S = 4 * i
        smask[I + S] = 1
        smask[L + S] = 1
        emask[I + S] = 1
        emask[Bt + S] = 1
    return tmask, smask, emask


def _build_nc():
    import concourse.bacc as bacc
    import concourse.mybir as mybir
    from concourse import tile

    f32 = mybir.dt.float32
    bf16 = mybir.dt.bfloat16
    AF = mybir.ActivationFunctionType

    nc = bacc.Bacc(None, target_bir_lowering=False, debug=False)
    em = nc.dram_tensor("em", [NBLK, ROWS, KSTEP, NG * BC], bf16,
                        kind="ExternalInput")
    w = nc.dram_tensor("w", [ROWS, MAUG], f32, kind="ExternalInput")
    selm = nc.dram_tensor("selm", [NV, ROWS], f32, kind="ExternalInput")
    en_out = [
        nc.dram_tensor(f"en{g}", [NBLK, NV, KSTEP, BC], f32,
                       kind="ExternalOutput")
        for g in range(NG)
    ]

    with tile.TileContext(nc) as tc:
        with (
            tc.tile_pool(name="const", bufs=1) as constp,
            tc.tile_pool(name="xraw", bufs=3) as xrawp,
            tc.tile_pool(name="xexp", bufs=3) as xexpp,
            tc.tile_pool(name="state", bufs=STATE_BUFS) as statep,
            tc.tile_pool(name="small", bufs=3) as smallp,
            tc.tile_pool(name="psA", bufs=PS_BUFS, space="PSUM") as psA,
            tc.tile_pool(name="psB", bufs=PS_BUFS, space="PSUM") as psB,
            tc.tile_pool(name="psR", bufs=PSR_BUFS, space="PSUM") as psR,
        ):
            wt = constp.tile([ROWS, MAUG], f32)
            nc.sync.dma_start(wt[:], w[:])
            # selector for broadcasting the two per-chain rescale rows down
            # to their 41-row blocks: rows 96/97, cols = chain row ranges
            sel = constp.tile([NV, ROWS], f32)
            nc.sync.dma_start(sel[:], selm[:])

            pspools = [psA, psB]

            def make_xe(q):
                xr = xrawp.tile([ROWS, KSTEP, NG * BC], bf16, tag="xr",
                                name="xr")
                nc.sync.dma_start(xr[:], em[q])
                xe = xexpp.tile([ROWS, KSTEP, NG * BC], f32, tag="xe",
                                name="xe")
                nc.scalar.activation(xe[:], xr[:], AF.Exp)
                return xe

            state = [None] * NG
            xe_cur = make_xe(0)
            for q in range(NBLK):
                xe_next = make_xe(q + 1) if q + 1 < NBLK else None
                ps = [pspools[g].tile([MAUG, KSTEP, BC], f32, tag=f"ps{g}",
                                      name=f"ps{g}")
                      for g in range(NG)]
                for u in range(KSTEP):
                    t = KSTEP * q + u + 1
                    for g in range(NG):
                        rhs = (state[g] if t > 1
                               else xe_cur[:, 0, g * BC:(g + 1) * BC])
                        nc.tensor.matmul(ps[g][:, u, :], wt[:], rhs)
                        if t <= T - 1:
                            xs = (xe_cur if u < KSTEP - 1 else xe_next)
                            ux = (u + 1) % KSTEP
                            newst = statep.tile([ROWS, BC], f32, tag=f"st{g}",
                                                name=f"st{g}")
                            nc.vector.tensor_mul(
                                newst[:],
                                ps[g][0:ROWS, u, :],
                                xs[:, ux, g * BC:(g + 1) * BC],
                            )
                            state[g] = newst
                            if t == 1 and g == 0:
                                # one-off DVE op (~half a step round-trip) to
                                # push the two groups into anti-phase; with a
                                # symmetric start they lock in-phase and the
                                # serial MM->mul->MM latency is unhidden.
                                dmy = smallp.tile([ROWS, 4, BC], f32,
                                                  tag="dmy", name="dmy")
                                nc.vector.tensor_mul(
                                    dmy[:], xe_cur[:, 0:4, 0:BC],
                                    xe_cur[:, 0:4, BC:2 * BC],
                                )
                        if DO_RESCALE and t % RESCALE_EVERY == 4 and t + 4 <= T - 1:
                            # rescale divisor = endsum(A_{8q+3}); reciprocal
                            # here, broadcast via PE, applied to the exp'd
                            # emissions of step 8q+8 (slot 0 of next block).
                            # Host recovers the log from the EN stream.
                            rsm = smallp.tile([NV, BC], f32,
                                              tag=f"rsm{g}", name=f"rsm{g}")
                            nc.vector.reciprocal(
                                rsm[:],
                                ps[g][AUX0:AUX0 + NV, u, :],
                            )
                            rcb = psR.tile([ROWS, BC], f32, tag=f"rcb{g}",
                                           name=f"rcb{g}")
                            nc.tensor.matmul(rcb[:], sel[:], rsm[:])
                            ta = t + 4
                            xa = xe_cur if ta // KSTEP == q else xe_next
                            ua = ta % KSTEP
                            nc.vector.tensor_mul(
                                xa[:, ua, g * BC:(g + 1) * BC],
                                xa[:, ua, g * BC:(g + 1) * BC],
                                rcb[:],
                            )
                for g in range(NG):
                    enst = smallp.tile([NV, KSTEP, BC], f32,
                                       tag=f"en{g}", name=f"en{g}")
                    nc.scalar.activation(
                        enst[:], ps[g][AUX0:AUX0 + NV, :, :], AF.Copy,
                    )
                    nc.sync.dma_start(en_out[g][q], enst[:])
                xe_cur = xe_next
    nc.compile()
    return nc


def _get_compiled():
    if "nc" not in _CACHE:
        _CACHE["nc"] = _build_nc()
    return _CACHE["nc"]


def kernel(emissions, mask, tags, transitions, start_transitions,
           end_transitions):
    import os
    import ml_dtypes
    from concourse.bass_utils import run_bass_kernel_spmd

    emissions = np.ascontiguousarray(np.asarray(emissions, dtype=np.float32))
    mask = np.asarray(mask).astype(bool)
    tags = np.asarray(tags).astype(np.int64)

    tmask, smask, emask = _bioul_masks()
    transC = np.where(tmask, IMPOSSIBLE, np.asarray(transitions, np.float32)).astype(np.float32)
    startC = np.where(smask, IMPOSSIBLE, np.asarray(start_transitions, np.float32)).astype(np.float32)
    endC = np.where(emask, IMPOSSIBLE, np.asarray(end_transitions, np.float32)).astype(np.float32)

    E = np.exp(transC)
    eend = np.exp(endC)
    W = np.zeros((ROWS, MAUG), np.float32)
    W[0:K, 0:K] = E
    W[K + 1:ROWS, K + 1:ROWS] = E
    W[0:K, AUX0] = eend
    W[K + 1:ROWS, AUX0 + 1] = eend

    # [B,T,K] -> per-core [NBLK, ROWS, KSTEP, NG*BC]
    # lane (c, g, v, b): batch = c*128 + g*64 + v*32 + b
    em_c = emissions - np.float32(MU)
    em_c[:, 0, :] += startC[None, :]
    emr = em_c.reshape(NCORES, NG, NV, BC, NBLK, KSTEP, K)
    em_r = np.zeros((NCORES, NBLK, ROWS, KSTEP, NG * BC), np.float32)
    for v in range(NV):
        # (c,g,b,q,u,j) -> (c,q,j,u,(g,b))
        blk = emr[:, :, v].transpose(0, 3, 5, 4, 1, 2)
        em_r[:, :, 42 * v:42 * v + K] = blk.reshape(
            NCORES, NBLK, K, KSTEP, NG * BC)
    import ml_dtypes as _md
    em_r = em_r.astype(_md.bfloat16)

    selm = np.zeros((NV, ROWS), np.float32)
    selm[0, 0:K] = 1.0
    selm[1, K + 1:ROWS] = 1.0

    nc = _get_compiled()
    in_maps = [{"em": em_r[c], "w": W, "selm": selm} for c in range(NCORES)]
    out = run_bass_kernel_spmd(
        nc, in_maps, list(range(NCORES)),
        trace=os.environ.get("CRF_TRACE", "") == "1",
    )
    _CACHE["exec_time_ns"] = out.exec_time_ns
    _CACHE["profile_json"] = out.profile_json
    res = out.results

    # EN[t, lane] = endsum(A_t); assemble z = log(EN[len-1]) + S[(len-1)//8]
    EN = np.zeros((B, T), np.float32)
    for c in range(NCORES):
        for g in range(NG):
            en = res[c][f"en{g}"]                     # [NBLK, NV, KSTEP, BC]
            for v in range(NV):
                gsl = slice(c * BLOC + g * (NV * BC) + v * BC,
                            c * BLOC + g * (NV * BC) + (v + 1) * BC)
                EN[gsl] = en[:, v].reshape(T, BC).T

    # rescale divisors: endsum(A_t) at t = 32q'+3 (q'=0..31), applied at
    # step 32q'+8 = slot 0 of 8-step block 4q'+1
    logs = np.log(EN[:, 3::RESCALE_EVERY].astype(np.float64))        # [B,32]
    cums = np.cumsum(logs, axis=1)                                   # [B,32]
    S = np.zeros((B, NBLK))
    S[:, 1:] = np.repeat(cums, 4, axis=1)[:, :NBLK - 1]

    lens = mask.sum(1).astype(np.int64)
    tstar = lens - 1
    bidx = np.arange(B)
    z = (np.log(EN[bidx, tstar].astype(np.float64)) + S[bidx, tstar // KSTEP]
         + MU * (tstar + 1))

    # gold-path score on host (f64)
    tC, sC, eC = (transC.astype(np.float64), startC.astype(np.float64),
                  endC.astype(np.float64))
    em_path = np.take_along_axis(emissions, tags[:, :, None], 2)[:, :, 0].astype(np.float64)
    t_last = tags[bidx, tstar]
    score = (sC[tags[:, 0]] + em_path[:, 0]
             + (mask[:, 1:] * (tC[tags[:, :-1], tags[:, 1:]] + em_path[:, 1:])).sum(1)
             + eC[t_last])
    return np.float32((score - z).mean())
